# revision 25
# baseline (speedup 1.0000x reference)
"""Causal self-attention (GPT-2 block) for Trainium2, 8 NeuronCores.

Sharding: core = 2*batch + head_group. Each of the 8 cores handles one of
B=4 batches and one group of 8 of the 16 heads (Megatron column-split of
the QKV weights, row-split of the proj weights). The two head-group
partial proj outputs per batch are summed on the host; the V-bias and
proj-bias terms are folded into a single host-side additive correction
(softmax rows sum to 1, so attn @ (1 x bv) == bv broadcast).

On-core layout (bf16 matmul operands; PE at 1 cycle/row, single-pass
LDWEIGHTS — fp32r runs LOW/HIGH double passes and was measured 2x
slower):
  xT    [128, 8, S]   x transposed via PE transpose-mode; ALL 8 chunks
                      persist so K and Q production can consume x in
                      opposite orders.
  QT/KT [128, 4, S]   feature-major: partition p, slice j <-> feature
                      j*128+p; head h lives at partitions (h%2)*64,
                      slice h//2
  V     [128, 16, 8, 65]  natural [s, feat] per head + ones column
                      (row sums ride along in the PV matmul for free)
  attnT [128, 4, S]   attention output, feature-major (proj stationary)

Schedule: production window w in 0..7 emits K/V for x-chunk w (K blocks
kb=2w,2w+1) and Q for x-chunk 7-w (so QT for the LAST attention chunk
finishes after window 1 and attention spreads across the whole run
instead of piling into a half-clock tail; the PE HAM activity monitor
drops the clock 2.4->1.2 GHz when the dense-matmul fraction falls).
Attention for (pair, q-chunk) runs in multiple PSUM passes over
contiguous k-block ranges as readiness allows; each pass drains into a
bf16 SBUF accumulator (DVE add), and the last pass triggers the
normalization (DMA-scattered reciprocal + DRAM-bounce broadcast).

Per attention block (128 k x 512 q): scoresT for the head PAIR goes into
one 2-bank PSUM tile ([128, 2, 512]) so a single ScalarE exp covers both
heads (ACT per-instruction overhead halved); the causal corner of diag
blocks is zeroed post-exp by a 0/1 bf16 mask multiply on the otherwise
idle Pool engine (GPSIMD cannot touch PSUM, but sx lives in SBUF).
"""

import os

import numpy as np

import concourse.bass as bass
import concourse.tile as tile
from concourse import bacc, mybir
from concourse.bass_utils import run_bass_kernel_spmd
from concourse.masks import make_identity, make_upper_triangular

# Problem shape (fixed by the harness contract).
B, S, D, H, HD = 4, 2048, 1024, 16, 64
NCORES = 8
HG = 8                # heads per core
FG = HG * HD          # 512 features per head group
P = 128
DB = D // P           # 8 contraction blocks
FBN = FG // P         # 4 feature blocks
SC = 512              # attention sequence chunk
NQ = S // SC          # 4
NKB = S // P          # 16 key blocks
NPAIR = HG // 2       # 4 head pairs
F32 = mybir.dt.float32
F32R = mybir.dt.float32r
BF16 = mybir.dt.bfloat16
DT_MM = BF16 if os.environ.get("KERNEL_DT", "bf16") == "bf16" else F32R
EXP = mybir.ActivationFunctionType.Exp
SCALE = 1.0 / float(HD) ** 0.5


class _Ctx:
    """Tiles/pools shared by the emission thunks."""


def _attention_pass_thunks(nc, cx, pair, q, kbs, first, last):
    """Thunks for one PSUM pass of attention: head-pair `pair`, q-chunk
    `q`, k-blocks `kbs` (contiguous). `first` pass drains by copy, later
    passes accumulate into the SBUF raw tiles; `last` appends the
    normalization."""
    hA, hB = 2 * pair, 2 * pair + 1
    nblk = len(kbs)
    st = {}

    def setup():
        st["heads"] = []
        for h in (hA, hB):
            out_ps = cx.psout.tile([65, SC], F32, tag="outps")
            st["heads"].append((h, (h % 2) * 64, h // 2, out_ps))

    def make_blk(i, kb):
        jj = kb - 4 * q if kb >= 4 * q else None

        def run():
            heads = st["heads"]
            off = 0 if jj is None else jj * P
            w = SC - off
            # Pair-shared PSUM tile (2 banks): one exp instruction covers
            # both heads, halving ACT per-instruction overhead.
            stp = cx.psst.tile([P, 2, SC], F32, tag="stps")
            for hi, (h, pb, j, out_ps) in enumerate(heads):
                nc.tensor.matmul(
                    stp[:, hi, :w],
                    cx.KT[pb:pb + 64, j, kb * P:(kb + 1) * P],
                    cx.QTc[q][pb:pb + 64, j, off:SC],
                    start=True, stop=True, tile_position=(pb, 0))
            sx = cx.sxp.tile([P, 2, SC], DT_MM, tag="sx")
            nc.scalar.activation(sx[:, :, :w], stp[:, :, :w], EXP,
                                 scale=SCALE)
            if jj is not None:
                for hi in range(2):
                    nc.gpsimd.tensor_mul(
                        sx[:, hi, 0:P], sx[:, hi, 0:P], cx.mask01)
            for hi, (h, pb, j, out_ps) in enumerate(heads):
                nc.tensor.matmul(
                    out_ps[:, off:], cx.V[:, kb, h, :], sx[:, hi, :w],
                    start=(i == 0), stop=(i == nblk - 1))
        return run

    def drain():
        for hi, (h, pb, j, out_ps) in enumerate(st["heads"]):
            raw = cx.raw[pair][q][hi]
            if first:
                nc.vector.tensor_copy(raw, out_ps)
            else:
                nc.vector.tensor_add(raw, raw, out_ps)

    thunks = [setup] + [make_blk(i, kb) for i, kb in enumerate(kbs)]
    thunks.append(drain)
    if last:
        def norm():
            for hi, (h, pb, j, _) in enumerate(st["heads"]):
                raw = cx.raw[pair][q][hi]
                # Single-partition reciprocal blocks the DVE FIFO for
                # ~us; DMA-scatter the sums across 128 partitions first.
                rshb = cx.nrmbc.tile([P, SC // P], DT_MM, tag="rshb")
                nc.sync.dma_start(rshb, raw[64:65, :])
                rsh = cx.nrmbc.tile([P, SC // P], F32, tag="rsh")
                nc.vector.tensor_copy(rsh, rshb)
                nc.vector.reciprocal(rsh, rsh)
                rdram = cx.drp.tile([1, SC], F32, tag="rdram")
                nc.sync.dma_start(rdram, rsh)
                rb = cx.nrmbc.tile([64, SC], F32, tag="rb")
                nc.sync.dma_start(rb, rdram.to_broadcast([64, SC]))
                nc.vector.tensor_mul(
                    cx.attnTc[q][pb:pb + 64, j, :], raw[0:64, :], rb)
        thunks.append(norm)
    return thunks


def _proj_chunk_thunks(nc, cx, q, out_d):
    """Proj for the s-blocks of chunk q; output DMAs straight from PSUM.
    These allocate the ps1 ring, so they must only be dripped BETWEEN
    production matmul groups (never inside one)."""
    thunks = []
    for sb in range(SC // P):
        sblk = q * (SC // P) + sb

        def make_half(hf, sblk=sblk, sb=sb):
            def run():
                ps = cx.ps1.tile([P, D // 2], F32, tag="qkps")
                n0 = hf * (D // 2)
                for j in range(FBN):
                    nc.tensor.matmul(
                        ps,
                        cx.attnTc[q][:, j, sb * P:(sb + 1) * P],
                        cx.wp_sb[:, j, n0:n0 + D // 2],
                        start=(j == 0), stop=(j == FBN - 1))
                og = cx.ogp.tile([P, D // 2], F32, tag="og")
                nc.vector.tensor_copy(og, ps)
                nc.sync.dma_start(
                    out_d.ap()[sblk * P:(sblk + 1) * P, n0:n0 + D // 2], og)
            return run

        thunks.append(make_half(0))
        thunks.append(make_half(1))
    return thunks


def _body(tc, x_d, wq_d, wk_d, wv_d, wp_d, bq_d, bk_d, out_d):
    nc = tc.nc
    cx = _Ctx()
    XC = 256                  # production chunk width (tokens)
    NXC = S // XC             # 8
    with (
        tc.tile_pool(name="persist", bufs=1) as persist,
        tc.tile_pool(name="ph1", bufs=1) as ph1,
        tc.tile_pool(name="xin", bufs=6) as xinp,
        tc.tile_pool(name="xtp", bufs=NXC) as xtp,
        tc.tile_pool(name="qtc", bufs=NQ) as qtc,
        tc.tile_pool(name="atc", bufs=NQ) as atc,
        tc.tile_pool(name="rawp", bufs=NPAIR * NQ * 2) as rawp,
        tc.tile_pool(name="sxp", bufs=3) as sxp,
        tc.tile_pool(name="nrmbc", bufs=2) as nrmbc,
        tc.tile_pool(name="ogp", bufs=2) as ogp,
        # PSUM banks: qkps/proj 1 + pt 1 + stps pair (2 banks x 2) 4
        # + outps 2 = 8
        tc.tile_pool(name="ps1", bufs=1, space="PSUM") as ps1,
        tc.tile_pool(name="ptp", bufs=1, space="PSUM") as ptp,
        tc.tile_pool(name="psst", bufs=2, space="PSUM") as psst,
        tc.tile_pool(name="psout", bufs=2, space="PSUM") as psout,
        tc.tile_pool(name="drp", bufs=8, space="DRAM") as drp,
    ):
        cx.sxp, cx.nrmbc, cx.ps1, cx.ogp = sxp, nrmbc, ps1, ogp
        cx.psst, cx.psout, cx.drp = psst, psout, drp

        ident = persist.tile([P, P], DT_MM)
        make_identity(nc, ident)
        for _ in range(12):
            jp = ptp.tile([P, P], F32, tag="pt")
            nc.tensor.matmul(jp, ident, ident, start=True, stop=True)
        # 0/1 multiplicative causal mask (valid where q-col >= k-row).
        cx.mask01 = persist.tile([P, P], DT_MM)
        make_upper_triangular(nc, cx.mask01, val=1.0, diag=True)
        bq_sb = persist.tile([P, FBN], F32)
        bk_sb = persist.tile([P, FBN], F32)
        nc.sync.dma_start(bq_sb, bq_d.ap().rearrange("(j p) -> p j", p=P))
        nc.sync.dma_start(bk_sb, bk_d.ap().rearrange("(j p) -> p j", p=P))

        cx.KT = persist.tile([P, FBN, S], DT_MM)
        cx.V = persist.tile([P, NKB, HG, HD + 1], DT_MM)
        ones_col = persist.tile([P, 1], F32)
        nc.vector.memset(ones_col, 1.0)
        nc.vector.tensor_copy(cx.V[:, :, :, HD],
                              ones_col.to_broadcast([P, NKB, HG]))
        cx.wp_sb = persist.tile([P, FBN, D], DT_MM)
        cx.QTc = [qtc.tile([P, FBN, SC], DT_MM, tag="qtc", name=f"qtc{q}")
                  for q in range(NQ)]
        cx.attnTc = [atc.tile([P, FBN, SC], DT_MM, tag="atc",
                              name=f"atc{q}") for q in range(NQ)]
        # bf16 SBUF accumulators for multi-pass PV (row 64 = rowsums).
        cx.raw = [[[rawp.tile([65, SC], DT_MM, tag="raw",
                              name=f"raw{p}_{q}_{hi}") for hi in range(2)]
                   for q in range(NQ)] for p in range(NPAIR)]

        wq_sb = ph1.tile([P, DB, FG], DT_MM)
        wk_sb = ph1.tile([P, DB, FG], DT_MM)
        wv_sb = ph1.tile([P, DB, FG], DT_MM)

        xts = [xtp.tile([P, DB, XC], DT_MM, tag="xt", name=f"xt{xc}")
               for xc in range(NXC)]

        def transpose_chunk(xc, warm=False):
            xt = xts[xc]
            thunks = []
            for sb in range(XC // P):
                s0 = xc * XC + sb * P
                for dh in range(2):
                    xin = xinp.tile([P, D // 2], DT_MM, tag="xin")
                    nc.sync.dma_start(
                        xin, x_d.ap()[s0:s0 + P,
                                      dh * (D // 2):(dh + 1) * (D // 2)])
                    if warm:
                        # Paced pre-warm junk matmuls keyed to the input
                        # DMAs keep the PE HAM busy through the initial
                        # load window. Must NOT share the 1-deep ptp ring
                        # with the transposes (WAW cycle via the xin ring).
                        wp_ps = ps1.tile([P, P], F32, tag="qkps")
                        nc.tensor.matmul(wp_ps, ident, xin[:, 0:P],
                                         start=True, stop=True)
                    for db4 in range(DB // 2):
                        db = dh * (DB // 2) + db4
                        def t(sb=sb, db=db, db4=db4, xin=xin, xt=xt):
                            pt = ptp.tile([P, P], DT_MM, tag="pt")
                            nc.tensor.transpose(
                                pt, xin[:, db4 * P:(db4 + 1) * P], ident)
                            nc.vector.tensor_copy(
                                xt[:, db, sb * P:(sb + 1) * P], pt)
                        thunks.append(t)
            return thunks

        for w_sb, w_d in ((wq_sb, wq_d), (wk_sb, wk_d), (wv_sb, wv_d)):
            wr = w_d.ap().rearrange("(db p) f -> db p f", p=P)
            for db in range(DB):
                nc.sync.dma_start(w_sb[:, db], wr[db])
        nc.sync.dma_start(
            cx.wp_sb, wp_d.ap().rearrange("(j p) n -> p j n", p=P))

        # Pre-loop: transpose the first K chunk and the first Q chunk.
        for t in transpose_chunk(0, warm=True):
            t()
        for t in transpose_chunk(7, warm=True):
            t()

        # Remaining transposes drip during the first three windows, in
        # the order both production streams need them.
        TR_SCHED = {0: (1, 6), 1: (2, 5), 2: (3, 4)}

        # Attention pass readiness: pass (q, kbs) may start after window
        # max(7-2q, max(kb)//2).  first/last flags per (pair, q).
        PASS_TABLE = {
            1: [(3, range(0, 4), True, False)],
            3: [(3, range(4, 8), False, False),
                (2, range(0, 8), True, False)],
            5: [(3, range(8, 12), False, False),
                (2, range(8, 12), False, True),
                (1, range(0, 8), True, True)],
            7: [(3, range(12, 16), False, True),
                (0, range(0, 4), True, True)],
        }

        tr = []      # deferred transpose thunks (ptp ring — safe inside
                     # matmul groups)
        bga = []     # attention/norm thunks (psst/psout/sx — safe inside
                     # groups)
        bgp = []     # proj thunks (ps1 ring — ONLY between groups)

        def drip(ntr, nbga, nbgp=0):
            for _ in range(ntr):
                if tr:
                    tr.pop(0)()
            for _ in range(nbga):
                if bga:
                    bga.pop(0)()
            for _ in range(nbgp):
                if bgp:
                    bgp.pop(0)()

        for w in range(NXC):
            if w in TR_SCHED:
                for c in TR_SCHED[w]:
                    tr += transpose_chunk(c)
            per_a = max(1, min(4, (len(bga) + 9) // 10))
            per_p = 1 if bgp else 0
            kc = w               # K/V source chunk (forward)
            qc = NXC - 1 - w     # Q source chunk (reverse)
            qq, qhalf = divmod(qc, 2)

            for w_sb, Tc, b_sb, src in ((wk_sb, None, bk_sb, kc),
                                        (wq_sb, cx.QTc, bq_sb, qc)):
                xt = xts[src]
                for fb in range(FBN):
                    ps = ps1.tile([P, XC], F32, tag="qkps")
                    for db in range(DB):
                        nc.tensor.matmul(
                            ps,
                            w_sb[:, db, fb * P:(fb + 1) * P],
                            xt[:, db, :],
                            start=(db == 0), stop=(db == DB - 1))
                        drip(1 if db % 2 else 0, 1 if db % 2 else 0)
                    if Tc is None:
                        dst = cx.KT[:, fb, kc * XC:(kc + 1) * XC]
                    else:
                        dst = Tc[qq][:, fb, qhalf * XC:(qhalf + 1) * XC]
                    nc.vector.tensor_scalar_add(dst, ps, b_sb[:, fb:fb + 1])
                    drip(0, per_a, per_p)
            xt = xts[kc]
            for sb in range(XC // P):
                kb = kc * (XC // P) + sb
                ps = ps1.tile([P, FG], F32, tag="qkps")
                for db in range(DB):
                    nc.tensor.matmul(
                        ps,
                        xt[:, db, sb * P:(sb + 1) * P],
                        wv_sb[:, db, :],
                        start=(db == 0), stop=(db == DB - 1))
                    drip(1 if db % 2 else 0, 1 if db % 2 else 0)
                nc.vector.tensor_copy(
                    cx.V[:, kb, :, 0:HD],
                    ps.rearrange("p (h c) -> p h c", h=HG))
                drip(0, per_a, per_p)
            while tr:
                tr.pop(0)()

            for (q, kbs, first, last) in PASS_TABLE.get(w, []):
                for p in range(NPAIR):
                    bga += _attention_pass_thunks(nc, cx, p, q, list(kbs),
                                                  first, last)
                if last:
                    # Sentinel: enqueue proj only once every pair's norm
                    # for this chunk has been EMITTED (FIFO guarantees the
                    # norms above pop first), so proj's attnTc reads are
                    # ordered after the norm writes.
                    def mk_sentinel(q=q):
                        def s():
                            bgp.extend(_proj_chunk_thunks(nc, cx, q, out_d))
                        return s
                    bga.append(mk_sentinel())

        # Tail: drain the queues; junk full-array matmuls keep the PE HAM
        # activity monitor above the half-clock threshold while the
        # attention/norm stragglers (ACT/DVE/DMA-bound) finish.
        def junk():
            jp = ptp.tile([P, P], F32, tag="pt")
            nc.tensor.matmul(jp, ident, ident, start=True, stop=True)

        while bga or bgp:
            if bga:
                bga.pop(0)()
            junk()
            if bgp:
                bgp.pop(0)()


def build_nc():
    nc = bacc.Bacc("TRN2", target_bir_lowering=False)
    x_d = nc.dram_tensor("x", [S, D], DT_MM, kind="ExternalInput")
    wq_d = nc.dram_tensor("wq", [D, FG], DT_MM, kind="ExternalInput")
    wk_d = nc.dram_tensor("wk", [D, FG], DT_MM, kind="ExternalInput")
    wv_d = nc.dram_tensor("wv", [D, FG], DT_MM, kind="ExternalInput")
    wp_d = nc.dram_tensor("wp", [FG, D], DT_MM, kind="ExternalInput")
    bq_d = nc.dram_tensor("bq", [FG], F32, kind="ExternalInput")
    bk_d = nc.dram_tensor("bk", [FG], F32, kind="ExternalInput")
    out_d = nc.dram_tensor("out", [S, D], F32, kind="ExternalOutput")
    with tile.TileContext(nc) as tc:
        _body(tc, x_d, wq_d, wk_d, wv_d, wp_d, bq_d, bk_d, out_d)
    nc.compile()
    return nc


_NC = None


def _get_nc():
    global _NC
    if _NC is None:
        _NC = build_nc()
    return _NC


def make_in_maps(hs, w, bvec, pw):
    import ml_dtypes
    wdt = ml_dtypes.bfloat16 if DT_MM == BF16 else np.float32
    in_maps = []
    for core in range(NCORES):
        b, g = divmod(core, 2)
        lo, hi = g * FG, (g + 1) * FG
        in_maps.append({
            "x": np.ascontiguousarray(hs[b]).astype(wdt),
            "wq": np.ascontiguousarray(w[:, lo:hi]).astype(wdt),
            "wk": np.ascontiguousarray(w[:, D + lo:D + hi]).astype(wdt),
            "wv": np.ascontiguousarray(
                w[:, 2 * D + lo:2 * D + hi]).astype(wdt),
            "wp": np.ascontiguousarray(pw[lo:hi, :]).astype(wdt),
            "bq": np.ascontiguousarray(bvec[lo:hi]),
            "bk": np.ascontiguousarray(bvec[D + lo:D + hi]),
        })
    return in_maps


def combine(parts, bvec, pw, pb):
    bv = bvec[2 * D:3 * D].astype(np.float64)
    corr = (bv @ pw.astype(np.float64) + pb.astype(np.float64)).astype(
        np.float32)
    out = np.empty((B, S, D), np.float32)
    for b in range(B):
        out[b] = parts[2 * b] + parts[2 * b + 1] + corr
    return out


def kernel(hidden_states, c_attn_w, c_attn_b, c_proj_w, c_proj_b,
           **run_kwargs):
    hs = np.asarray(hidden_states, dtype=np.float32)
    w = np.asarray(c_attn_w, dtype=np.float32)
    bvec = np.asarray(c_attn_b, dtype=np.float32)
    pw = np.asarray(c_proj_w, dtype=np.float32)
    pb = np.asarray(c_proj_b, dtype=np.float32)
    nc = _get_nc()
    res = run_bass_kernel_spmd(nc, make_in_maps(hs, w, bvec, pw),
                               core_ids=list(range(NCORES)), **run_kwargs)
    parts = [res.results[i]["out"] for i in range(NCORES)]
    out = combine(parts, bvec, pw, pb)
    if run_kwargs:
        return out, res
    return out


# revision 27
# speedup vs baseline: 1.0468x; 1.0468x over previous
"""Causal self-attention (GPT-2 block) for Trainium2, 8 NeuronCores.

Sharding: core = 2*batch + head_group. Each of the 8 cores handles one of
B=4 batches and one group of 8 of the 16 heads (Megatron column-split of
the QKV weights, row-split of the proj weights). The two head-group
partial proj outputs per batch are summed on the host; the V-bias and
proj-bias terms are folded into a single host-side additive correction
(softmax rows sum to 1, so attn @ (1 x bv) == bv broadcast).

On-core layout (bf16 matmul operands; PE at 1 cycle/row, single-pass
LDWEIGHTS — fp32r runs LOW/HIGH double passes and was measured 2x
slower):
  xT    [128, 8, S]   x transposed via PE transpose-mode; ALL 8 chunks
                      persist so K and Q production can consume x in
                      opposite orders.
  QT/KT [128, 4, S]   feature-major: partition p, slice j <-> feature
                      j*128+p; head h lives at partitions (h%2)*64,
                      slice h//2
  V     [128, 16, 8, 65]  natural [s, feat] per head + ones column
                      (row sums ride along in the PV matmul for free)
  attnT [128, 4, S]   attention output, feature-major (proj stationary)

Schedule: production window w in 0..7 emits K/V for x-chunk w (K blocks
kb=2w,2w+1) and Q for x-chunk 7-w (so QT for the LAST attention chunk
finishes after window 1 and attention spreads across the whole run
instead of piling into a half-clock tail; the PE HAM activity monitor
drops the clock 2.4->1.2 GHz when the dense-matmul fraction falls).
Attention for (pair, q-chunk) runs in multiple PSUM passes over
contiguous k-block ranges as readiness allows; each pass drains into a
bf16 SBUF accumulator (DVE add), and the last pass triggers the
normalization (DMA-scattered reciprocal + DRAM-bounce broadcast).

Per attention block (128 k x 512 q): scoresT for the head PAIR goes into
one 2-bank PSUM tile ([128, 2, 512]) so a single ScalarE exp covers both
heads (ACT per-instruction overhead halved); the causal corner of diag
blocks is zeroed post-exp by a 0/1 bf16 mask multiply on the otherwise
idle Pool engine (GPSIMD cannot touch PSUM, but sx lives in SBUF).
"""

import os

import numpy as np

import concourse.bass as bass
import concourse.tile as tile
from concourse import bacc, mybir
from concourse.bass_utils import run_bass_kernel_spmd
from concourse.masks import make_identity, make_upper_triangular

# Problem shape (fixed by the harness contract).
B, S, D, H, HD = 4, 2048, 1024, 16, 64
NCORES = 8
HG = 8                # heads per core
FG = HG * HD          # 512 features per head group
P = 128
DB = D // P           # 8 contraction blocks
FBN = FG // P         # 4 feature blocks
SC = 512              # attention sequence chunk
NQ = S // SC          # 4
NKB = S // P          # 16 key blocks
NPAIR = HG // 2       # 4 head pairs
F32 = mybir.dt.float32
F32R = mybir.dt.float32r
BF16 = mybir.dt.bfloat16
DT_MM = BF16 if os.environ.get("KERNEL_DT", "bf16") == "bf16" else F32R
EXP = mybir.ActivationFunctionType.Exp
SCALE = 1.0 / float(HD) ** 0.5


class _Ctx:
    """Tiles/pools shared by the emission thunks."""


def _attention_pass_thunks(nc, cx, pair, q, kbs, first, last):
    """Thunks for one PSUM pass of attention: head-pair `pair`, q-chunk
    `q`, k-blocks `kbs` (contiguous). `first` pass drains by copy, later
    passes accumulate into the SBUF raw tiles; `last` appends the
    normalization."""
    hA, hB = 2 * pair, 2 * pair + 1
    nblk = len(kbs)
    st = {}

    def setup():
        st["heads"] = []
        for h in (hA, hB):
            out_ps = cx.psout.tile([65, SC], F32, tag="outps")
            st["heads"].append((h, (h % 2) * 64, h // 2, out_ps))

    def make_blk(i, kb):
        jj = kb - 4 * q if kb >= 4 * q else None

        def run():
            heads = st["heads"]
            off = 0 if jj is None else jj * P
            w = SC - off
            # Pair-shared PSUM tile (2 banks): one exp instruction covers
            # both heads, halving ACT per-instruction overhead.
            stp = cx.psst.tile([P, 2, SC], F32, tag="stps")
            for hi, (h, pb, j, out_ps) in enumerate(heads):
                nc.tensor.matmul(
                    stp[:, hi, :w],
                    cx.KT[pb:pb + 64, j, kb * P:(kb + 1) * P],
                    cx.QTc[q][pb:pb + 64, j, off:SC],
                    start=True, stop=True, tile_position=(pb, 0))
            sx = cx.sxp.tile([P, 2, SC], DT_MM, tag="sx")
            nc.scalar.activation(sx[:, :, :w], stp[:, :, :w], EXP,
                                 scale=SCALE)
            if jj is not None:
                for hi in range(2):
                    nc.gpsimd.tensor_mul(
                        sx[:, hi, 0:P], sx[:, hi, 0:P], cx.mask01)
            for hi, (h, pb, j, out_ps) in enumerate(heads):
                nc.tensor.matmul(
                    out_ps[:, off:], cx.V[:, kb, h, :], sx[:, hi, :w],
                    start=(i == 0), stop=(i == nblk - 1))
        return run

    def drain():
        for hi, (h, pb, j, out_ps) in enumerate(st["heads"]):
            raw = cx.raw[pair][q][hi]
            if first:
                nc.vector.tensor_copy(raw, out_ps)
            else:
                nc.vector.tensor_add(raw, raw, out_ps)

    thunks = [setup] + [make_blk(i, kb) for i, kb in enumerate(kbs)]
    thunks.append(drain)
    if last:
        def norm():
            for hi, (h, pb, j, _) in enumerate(st["heads"]):
                raw = cx.raw[pair][q][hi]
                # Single-partition reciprocal blocks the DVE FIFO for
                # ~us; DMA-scatter the sums across 128 partitions first.
                rshb = cx.nrmbc.tile([P, SC // P], DT_MM, tag="rshb")
                nc.sync.dma_start(rshb, raw[64:65, :])
                rsh = cx.nrmbc.tile([P, SC // P], F32, tag="rsh")
                nc.vector.tensor_copy(rsh, rshb)
                nc.vector.reciprocal(rsh, rsh)
                rdram = cx.drp.tile([1, SC], F32, tag="rdram")
                nc.sync.dma_start(rdram, rsh)
                rb = cx.nrmbc.tile([64, SC], F32, tag="rb")
                nc.sync.dma_start(rb, rdram.to_broadcast([64, SC]))
                nc.vector.tensor_mul(
                    cx.attnTc[q][pb:pb + 64, j, :], raw[0:64, :], rb)
        thunks.append(norm)
    return thunks


def _proj_chunk_thunks(nc, cx, q, out_d):
    """Proj for the s-blocks of chunk q; output DMAs straight from PSUM.
    These allocate the ps1 ring, so they must only be dripped BETWEEN
    production matmul groups (never inside one)."""
    thunks = []
    for sb in range(SC // P):
        sblk = q * (SC // P) + sb

        def make_half(hf, sblk=sblk, sb=sb):
            def run():
                ps = cx.ps1.tile([P, D // 2], F32, tag="qkps")
                n0 = hf * (D // 2)
                for j in range(FBN):
                    nc.tensor.matmul(
                        ps,
                        cx.attnTc[q][:, j, sb * P:(sb + 1) * P],
                        cx.wp_sb[:, j, n0:n0 + D // 2],
                        start=(j == 0), stop=(j == FBN - 1))
                og = cx.ogp.tile([P, D // 2], F32, tag="og")
                nc.vector.tensor_copy(og, ps)
                nc.sync.dma_start(
                    out_d.ap()[sblk * P:(sblk + 1) * P, n0:n0 + D // 2], og)
            return run

        thunks.append(make_half(0))
        thunks.append(make_half(1))
    return thunks


def _body(tc, x_d, wq_d, wk_d, wv_d, wp_d, bq_d, bk_d, out_d):
    nc = tc.nc
    cx = _Ctx()
    XC = 256                  # production chunk width (tokens)
    NXC = S // XC             # 8
    with (
        tc.tile_pool(name="persist", bufs=1) as persist,
        tc.tile_pool(name="ph1", bufs=1) as ph1,
        tc.tile_pool(name="xin", bufs=6) as xinp,
        tc.tile_pool(name="xtp", bufs=NXC) as xtp,
        tc.tile_pool(name="qtc", bufs=NQ) as qtc,
        tc.tile_pool(name="atc", bufs=NQ) as atc,
        tc.tile_pool(name="rawp", bufs=NPAIR * NQ * 2) as rawp,
        tc.tile_pool(name="sxp", bufs=3) as sxp,
        tc.tile_pool(name="nrmbc", bufs=2) as nrmbc,
        tc.tile_pool(name="ogp", bufs=2) as ogp,
        # PSUM banks: qkps/proj 1 + pt 1 + stps pair (2 banks x 2) 4
        # + outps 2 = 8
        tc.tile_pool(name="ps1", bufs=1, space="PSUM") as ps1,
        tc.tile_pool(name="ptp", bufs=1, space="PSUM") as ptp,
        tc.tile_pool(name="psst", bufs=2, space="PSUM") as psst,
        tc.tile_pool(name="psout", bufs=2, space="PSUM") as psout,
        tc.tile_pool(name="drp", bufs=8, space="DRAM") as drp,
    ):
        cx.sxp, cx.nrmbc, cx.ps1, cx.ogp = sxp, nrmbc, ps1, ogp
        cx.psst, cx.psout, cx.drp = psst, psout, drp

        ident = persist.tile([P, P], DT_MM)
        make_identity(nc, ident)
        for _ in range(12):
            jp = ptp.tile([P, P], F32, tag="pt")
            nc.tensor.matmul(jp, ident, ident, start=True, stop=True)
        # 0/1 multiplicative causal mask (valid where q-col >= k-row).
        cx.mask01 = persist.tile([P, P], DT_MM)
        make_upper_triangular(nc, cx.mask01, val=1.0, diag=True)
        bq_sb = persist.tile([P, FBN], F32)
        bk_sb = persist.tile([P, FBN], F32)
        nc.sync.dma_start(bq_sb, bq_d.ap().rearrange("(j p) -> p j", p=P))
        nc.sync.dma_start(bk_sb, bk_d.ap().rearrange("(j p) -> p j", p=P))

        cx.KT = persist.tile([P, FBN, S], DT_MM)
        cx.V = persist.tile([P, NKB, HG, HD + 1], DT_MM)
        ones_col = persist.tile([P, 1], F32)
        nc.vector.memset(ones_col, 1.0)
        nc.vector.tensor_copy(cx.V[:, :, :, HD],
                              ones_col.to_broadcast([P, NKB, HG]))
        cx.wp_sb = persist.tile([P, FBN, D], DT_MM)
        cx.QTc = [qtc.tile([P, FBN, SC], DT_MM, tag="qtc", name=f"qtc{q}")
                  for q in range(NQ)]
        cx.attnTc = [atc.tile([P, FBN, SC], DT_MM, tag="atc",
                              name=f"atc{q}") for q in range(NQ)]
        # bf16 SBUF accumulators for multi-pass PV (row 64 = rowsums).
        cx.raw = [[[rawp.tile([65, SC], DT_MM, tag="raw",
                              name=f"raw{p}_{q}_{hi}") for hi in range(2)]
                   for q in range(NQ)] for p in range(NPAIR)]

        wq_sb = ph1.tile([P, DB, FG], DT_MM)
        wk_sb = ph1.tile([P, DB, FG], DT_MM)
        wv_sb = ph1.tile([P, DB, FG], DT_MM)

        xts = [xtp.tile([P, DB, XC], DT_MM, tag="xt", name=f"xt{xc}")
               for xc in range(NXC)]

        def transpose_chunk(xc, warm=False):
            xt = xts[xc]
            thunks = []
            for sb in range(XC // P):
                s0 = xc * XC + sb * P
                for dh in range(2):
                    xin = xinp.tile([P, D // 2], DT_MM, tag="xin")
                    nc.sync.dma_start(
                        xin, x_d.ap()[s0:s0 + P,
                                      dh * (D // 2):(dh + 1) * (D // 2)])
                    if warm:
                        # Paced pre-warm junk matmuls keyed to the input
                        # DMAs keep the PE HAM busy through the initial
                        # load window. Must NOT share the 1-deep ptp ring
                        # with the transposes (WAW cycle via the xin ring).
                        wp_ps = ps1.tile([P, P], F32, tag="qkps")
                        nc.tensor.matmul(wp_ps, ident, xin[:, 0:P],
                                         start=True, stop=True)
                    for db4 in range(DB // 2):
                        db = dh * (DB // 2) + db4
                        def t(sb=sb, db=db, db4=db4, xin=xin, xt=xt):
                            pt = ptp.tile([P, P], DT_MM, tag="pt")
                            nc.tensor.transpose(
                                pt, xin[:, db4 * P:(db4 + 1) * P], ident)
                            nc.vector.tensor_copy(
                                xt[:, db, sb * P:(sb + 1) * P], pt)
                        thunks.append(t)
            return thunks

        for w_sb, w_d in ((wq_sb, wq_d), (wk_sb, wk_d), (wv_sb, wv_d)):
            wr = w_d.ap().rearrange("(db p) f -> db p f", p=P)
            for db in range(DB):
                nc.sync.dma_start(w_sb[:, db], wr[db])
        nc.sync.dma_start(
            cx.wp_sb, wp_d.ap().rearrange("(j p) n -> p j n", p=P))

        # Pre-loop: transpose the first K chunk inline; everything else
        # (including chunk 7, which window 0's Q groups need) drips
        # between production matmul groups.
        for t in transpose_chunk(0, warm=True):
            t()

        TR_SCHED = {0: (7, 1, 6), 1: (2, 5), 2: (3, 4)}

        # Production plan: K/V forward, Q reverse; x7's K/V is pulled
        # into window 6 so window 7 keeps dense Q matmuls as HAM clock
        # ballast while the chunk-3 diagonal attention drains.
        KV_SCHED = {0: (0,), 1: (1,), 2: (2,), 3: (3,), 4: (4,),
                    5: (5,), 6: (6, 7), 7: ()}
        Q_SCHED = {0: 7, 1: 6, 2: 5, 3: 4, 4: 3, 5: 2, 6: 1, 7: 0}

        # Attention pass readiness: pass (q, kbs) may start once QT_q and
        # KT/V for its k-blocks exist.  first/last flags per (pair, q).
        PASS_TABLE = {
            1: [(3, range(0, 4), True, False)],
            3: [(3, range(4, 8), False, False),
                (2, range(0, 8), True, False)],
            5: [(3, range(8, 12), False, False),
                (2, range(8, 12), False, True),
                (1, range(0, 8), True, True)],
            6: [(3, range(12, 16), False, True)],
            7: [(0, range(0, 4), True, True)],
        }

        tr = []      # deferred transpose thunks (ptp ring — safe inside
                     # matmul groups)
        bga = []     # attention/norm thunks (psst/psout/sx — safe inside
                     # groups)
        bgp = []     # proj thunks (ps1 ring — ONLY between groups)

        def drip(ntr, nbga, nbgp=0):
            for _ in range(ntr):
                if tr:
                    tr.pop(0)()
            for _ in range(nbga):
                if bga:
                    bga.pop(0)()
            for _ in range(nbgp):
                if bgp:
                    bgp.pop(0)()

        for w in range(NXC):
            if w in TR_SCHED:
                for c in TR_SCHED[w]:
                    tr += transpose_chunk(c)
            # Cap attention pops per drip point: the PE wait queue holds
            # only 4 dependency-blocked instructions before head-of-line
            # blocking sets in.
            per_a = max(1, min(2, (len(bga) + 39) // 40))
            per_p = 1 if bgp else 0

            def kq_group(w_sb, Tc, b_sb, src):
                xt = xts[src]
                qq, qhalf = divmod(src, 2)
                for fb in range(FBN):
                    ps = ps1.tile([P, XC], F32, tag="qkps")
                    for db in range(DB):
                        nc.tensor.matmul(
                            ps,
                            w_sb[:, db, fb * P:(fb + 1) * P],
                            xt[:, db, :],
                            start=(db == 0), stop=(db == DB - 1))
                        drip(1 if db % 2 else 0, 1 if db % 2 else 0)
                    if Tc is None:
                        dst = cx.KT[:, fb, src * XC:(src + 1) * XC]
                    else:
                        dst = Tc[qq][:, fb, qhalf * XC:(qhalf + 1) * XC]
                    nc.vector.tensor_scalar_add(dst, ps, b_sb[:, fb:fb + 1])
                    drip(0, per_a, per_p)

            def v_group(kc):
                xt = xts[kc]
                for sb in range(XC // P):
                    kb = kc * (XC // P) + sb
                    ps = ps1.tile([P, FG], F32, tag="qkps")
                    for db in range(DB):
                        nc.tensor.matmul(
                            ps,
                            xt[:, db, sb * P:(sb + 1) * P],
                            wv_sb[:, db, :],
                            start=(db == 0), stop=(db == DB - 1))
                        drip(1 if db % 2 else 0, 1 if db % 2 else 0)
                    nc.vector.tensor_copy(
                        cx.V[:, kb, :, 0:HD],
                        ps.rearrange("p (h c) -> p h c", h=HG))
                    drip(0, per_a, per_p)

            # K/V first so chunk 7's dripped transposes land before the
            # window-0 Q groups consume xts[7].
            for kc in KV_SCHED[w]:
                kq_group(wk_sb, None, bk_sb, kc)
                v_group(kc)
            kq_group(wq_sb, cx.QTc, bq_sb, Q_SCHED[w])
            while tr:
                tr.pop(0)()

            for (q, kbs, first, last) in PASS_TABLE.get(w, []):
                for p in range(NPAIR):
                    bga += _attention_pass_thunks(nc, cx, p, q, list(kbs),
                                                  first, last)
                if last:
                    # Sentinel: enqueue proj only once every pair's norm
                    # for this chunk has been EMITTED (FIFO guarantees the
                    # norms above pop first), so proj's attnTc reads are
                    # ordered after the norm writes.
                    def mk_sentinel(q=q):
                        def s():
                            bgp.extend(_proj_chunk_thunks(nc, cx, q, out_d))
                        return s
                    bga.append(mk_sentinel())

        # Tail: drain the queues; junk full-array matmuls keep the PE HAM
        # activity monitor above the half-clock threshold while the
        # attention/norm stragglers (ACT/DVE/DMA-bound) finish.
        def junk():
            jp = ptp.tile([P, P], F32, tag="pt")
            nc.tensor.matmul(jp, ident, ident, start=True, stop=True)

        while bga or bgp:
            if bga:
                bga.pop(0)()
            junk()
            if bgp:
                bgp.pop(0)()


def build_nc():
    nc = bacc.Bacc("TRN2", target_bir_lowering=False)
    x_d = nc.dram_tensor("x", [S, D], DT_MM, kind="ExternalInput")
    wq_d = nc.dram_tensor("wq", [D, FG], DT_MM, kind="ExternalInput")
    wk_d = nc.dram_tensor("wk", [D, FG], DT_MM, kind="ExternalInput")
    wv_d = nc.dram_tensor("wv", [D, FG], DT_MM, kind="ExternalInput")
    wp_d = nc.dram_tensor("wp", [FG, D], DT_MM, kind="ExternalInput")
    bq_d = nc.dram_tensor("bq", [FG], F32, kind="ExternalInput")
    bk_d = nc.dram_tensor("bk", [FG], F32, kind="ExternalInput")
    out_d = nc.dram_tensor("out", [S, D], F32, kind="ExternalOutput")
    with tile.TileContext(nc) as tc:
        _body(tc, x_d, wq_d, wk_d, wv_d, wp_d, bq_d, bk_d, out_d)
    nc.compile()
    return nc


_NC = None


def _get_nc():
    global _NC
    if _NC is None:
        _NC = build_nc()
    return _NC


def make_in_maps(hs, w, bvec, pw):
    import ml_dtypes
    wdt = ml_dtypes.bfloat16 if DT_MM == BF16 else np.float32
    in_maps = []
    for core in range(NCORES):
        b, g = divmod(core, 2)
        lo, hi = g * FG, (g + 1) * FG
        in_maps.append({
            "x": np.ascontiguousarray(hs[b]).astype(wdt),
            "wq": np.ascontiguousarray(w[:, lo:hi]).astype(wdt),
            "wk": np.ascontiguousarray(w[:, D + lo:D + hi]).astype(wdt),
            "wv": np.ascontiguousarray(
                w[:, 2 * D + lo:2 * D + hi]).astype(wdt),
            "wp": np.ascontiguousarray(pw[lo:hi, :]).astype(wdt),
            "bq": np.ascontiguousarray(bvec[lo:hi]),
            "bk": np.ascontiguousarray(bvec[D + lo:D + hi]),
        })
    return in_maps


def combine(parts, bvec, pw, pb):
    bv = bvec[2 * D:3 * D].astype(np.float64)
    corr = (bv @ pw.astype(np.float64) + pb.astype(np.float64)).astype(
        np.float32)
    out = np.empty((B, S, D), np.float32)
    for b in range(B):
        out[b] = parts[2 * b] + parts[2 * b + 1] + corr
    return out


def kernel(hidden_states, c_attn_w, c_attn_b, c_proj_w, c_proj_b,
           **run_kwargs):
    hs = np.asarray(hidden_states, dtype=np.float32)
    w = np.asarray(c_attn_w, dtype=np.float32)
    bvec = np.asarray(c_attn_b, dtype=np.float32)
    pw = np.asarray(c_proj_w, dtype=np.float32)
    pb = np.asarray(c_proj_b, dtype=np.float32)
    nc = _get_nc()
    res = run_bass_kernel_spmd(nc, make_in_maps(hs, w, bvec, pw),
                               core_ids=list(range(NCORES)), **run_kwargs)
    parts = [res.results[i]["out"] for i in range(NCORES)]
    out = combine(parts, bvec, pw, pb)
    if run_kwargs:
        return out, res
    return out
